# revision 1
# baseline (speedup 1.0000x reference)
"""NetVLAD forward kernel for 8 TRN2 NeuronCores (Bass/Tile).

Reference (per batch b of 32):
  s = x @ Wk + b         (1024, 64) logits;  softmax over k -> a
  v[d,k] = sum_n a[n,k] x[n,d] + (sum_n a[n,k]) * C[d,k]
  v /= ||v||_2 over d (per k);  out = flatten(v) / ||flatten(v)||_2

Sharding: data-parallel over batch B=32 across 8 cores (4 batches/core).
Wk, b, C replicated; no collectives; host concatenates outputs.

Layout tricks (all [64, *] work is packed two-to-a-tile onto 128 partitions):
  - mm1 logits for both 512-pixel groups share one PSUM tile [128, 512]
    (group g occupies partitions 64g..64g+63), one Exp covers both.
  - mm2 vT / asum for a PAIR of batches share [128, 512] / [128, 1] PSUM.
  - softmax normalization is folded into x (x~ = x * 1/Z, per-partition
    scalar), so matmul2 operands are raw exp(e) and x~.
  - the norm tail (sqrt/recip/global-norm) is batched across all 4 batches
    at the end: ACT function switches (table loads ~1.3us each) drop to 2.
Engines: PE transposes+matmuls (bf16, fp32 PSUM accum), ACT = Exp + PSUM
copies, DVE = reductions/reciprocal/scaling, GPSIMD = f32->bf16 casting DMAs
only (its tensor ops are ~25x slower and starve DVE via shared SBUF ports).
"""

import sys

sys.path.insert(0, "/opt/trn_rl_repo")

from contextlib import ExitStack

import numpy as np

import concourse.bacc as bacc
import concourse.tile as tile
from concourse import mybir
from concourse.bass_utils import run_bass_kernel_spmd

F32 = mybir.dt.float32
BF16 = mybir.dt.bfloat16
AX = mybir.AxisListType
OP = mybir.AluOpType
ACTF = mybir.ActivationFunctionType

B_PER_CORE = 4  # 32 batches / 8 cores
N = 1024  # H*W pixels per batch
D = 512
K = 64
EPS = 1e-12
N_CORES = 8


def build_kernel():
    nc = bacc.Bacc()
    x = nc.declare_dram_parameter("x", [B_PER_CORE * N, D], F32, isOutput=False)
    out = nc.declare_dram_parameter("out", [B_PER_CORE, D * K], F32, isOutput=True)
    idbf_d = nc.declare_dram_parameter("idbf", [128, 128], BF16, isOutput=False)
    idf_d = nc.declare_dram_parameter("idf", [128, 128], F32, isOutput=False)
    wkb_d = nc.declare_dram_parameter("wkb", [128, 4, K], BF16, isOutput=False)
    ct2_d = nc.declare_dram_parameter("ct2", [128, D], F32, isOutput=False)
    b2_d = nc.declare_dram_parameter("b2", [128, 1], F32, isOutput=False)
    iddbl_d = nc.declare_dram_parameter("iddbl", [128, K], F32, isOutput=False)

    with tile.TileContext(nc) as tc, ExitStack() as ctx:
        const = ctx.enter_context(tc.tile_pool(name="const", bufs=1))
        xpool = ctx.enter_context(tc.tile_pool(name="xpool", bufs=4))
        xts = ctx.enter_context(tc.tile_pool(name="xts", bufs=6))
        sbm = ctx.enter_context(tc.tile_pool(name="sbm", bufs=2))
        nrm = ctx.enter_context(tc.tile_pool(name="nrm", bufs=2))
        # PSUM pools: xt2 + e2 + s2 + v1 + o1(shared asum/out) = 8 banks
        ps_xt = ctx.enter_context(tc.tile_pool(name="ps_xt", bufs=2, space="PSUM"))
        ps_e = ctx.enter_context(tc.tile_pool(name="ps_e", bufs=2, space="PSUM"))
        ps_s = ctx.enter_context(tc.tile_pool(name="ps_s", bufs=2, space="PSUM"))
        ps_v = ctx.enter_context(tc.tile_pool(name="ps_v", bufs=1, space="PSUM"))
        ps_o = ctx.enter_context(tc.tile_pool(name="ps_o", bufs=1, space="PSUM"))
        
        # ---- constants (host-prepared, loaded via HWDGE in parallel with x) ----
        id_bf = const.tile([128, 128], BF16)
        nc.sync.dma_start(out=id_bf[:], in_=idbf_d[:])
        id_f32 = const.tile([128, 128], F32)
        nc.sync.dma_start(out=id_f32[:], in_=idf_d[:])
        wkb = const.tile([128, 4, K], BF16)
        nc.sync.dma_start(out=wkb[:], in_=wkb_d[:])
        ct2 = const.tile([128, D], F32)
        nc.sync.dma_start(out=ct2[:], in_=ct2_d[:])
        b2_sb = const.tile([128, 1], F32)
        nc.sync.dma_start(out=b2_sb[:], in_=b2_d[:])
        eps64_sb = const.tile([128, 1], F32)
        nc.vector.memset(eps64_sb[:], float(64 * EPS))
        iddbl = const.tile([128, K], F32)
        nc.sync.dma_start(out=iddbl[:], in_=iddbl_d[:])

        # ---- per-batch pipeline ----
        v2 = {}
        S_all = nrm.tile([128, 2], F32, tag="sall")
        for b in range(B_PER_CORE):
            p, h = b // 2, b % 2

            xg = []
            for g in range(2):
                t = xpool.tile([128, 4, D], BF16, tag=f"xb{g}")
                src_ap = x[b * N + 512 * g : b * N + 512 * (g + 1), :].rearrange(
                    "(i p) d -> p i d", p=128
                )
                if b <= 1:
                    # split the cold-start load by d-halves so the j=0,1
                    # transposes can begin after the first 512KB lands
                    nc.gpsimd.dma_start(out=t[:, :, 0:256], in_=src_ap[:, :, 0:256])
                    nc.gpsimd.dma_start(out=t[:, :, 256:512], in_=src_ap[:, :, 256:512])
                else:
                    nc.gpsimd.dma_start(out=t[:], in_=src_ap)
                xg.append(t)

            # -- mm1 for both groups into one PSUM tile [128, 512] --
            # two d-chunks share one PSUM bank ([128,2,512]bf16 = 2KB) so a
            # single ACT copy feeds two mm1 matmuls (halves copy overhead)
            s_ps = ps_s.tile([128, 512], F32, tag="s")
            for g in range(2):
                for jj in range(2):  # d-chunk pairs
                    xt_ps = ps_xt.tile([128, 2, 512], BF16, tag="xt")
                    for j2 in range(2):
                        j = 2 * jj + j2
                        for c in range(4):  # n-subtiles
                            nc.tensor.transpose(
                                xt_ps[:, j2, c * 128 : (c + 1) * 128],
                                xg[g][:, c, j * 128 : (j + 1) * 128],
                                id_bf[:],
                            )
                    xt_sb = xts.tile([128, 2, 512], BF16, tag="xt_sb")
                    nc.scalar.copy(xt_sb[:], xt_ps[:])
                    for j2 in range(2):
                        j = 2 * jj + j2
                        nc.tensor.matmul(
                            s_ps[K * g : K * (g + 1), :],
                            wkb[:, j, :],
                            xt_sb[:, j2, :],
                            start=(j == 0),
                            stop=(j == 3),
                            skip_group_check=True,
                        )

            # -- exp(s + b) for both groups at once --
            eT = sbm.tile([128, 512], BF16, tag="eT")
            nc.scalar.activation(eT[:], s_ps[:], ACTF.Exp, bias=b2_sb[:])

            # -- transpose e back to [n, k]; Z; invZ --
            a_sb = sbm.tile([128, 8, K], BF16, tag="a")
            z_all = sbm.tile([128, 8], F32, tag="z")
            invz = sbm.tile([128, 8], F32, tag="invz")
            invz_bf = sbm.tile([128, 8], BF16, tag="invzbf")
            for g in range(2):
                e_ps = ps_e.tile([128, 4, K], BF16, tag="e")
                for c in range(4):
                    nc.tensor.transpose(
                        e_ps[:, c, :],
                        eT[K * g : K * (g + 1), c * 128 : (c + 1) * 128],
                        id_bf[K * g : K * (g + 1), K * g : K * (g + 1)],
                    )
                nc.vector.reduce_sum(z_all[:, g * 4 : (g + 1) * 4], e_ps[:], axis=AX.X)
                nc.vector.tensor_copy(a_sb[:, 4 * g : 4 * (g + 1), :], e_ps[:])
            nc.vector.reciprocal(invz[:], z_all[:])
            nc.vector.tensor_copy(invz_bf[:], invz[:])

            # -- x~ = x * invZ (per-pixel softmax denominator folded into x) --
            xsg = []
            for g in range(2):
                t = xpool.tile([128, 4, D], BF16, tag=f"xs{g}")
                for c in range(4):
                    i = 4 * g + c
                    nc.vector.tensor_scalar_mul(
                        t[:, c, :], xg[g][:, c, :], invz[:, i : i + 1]
                    )
                xsg.append(t)

            # -- mm2 + asum for the batch pair into [128, *] PSUM --
            if h == 0:
                v_ps = ps_v.tile([128, 512], F32, tag="v")
                as_ps = ps_o.tile([128, 1], F32, tag="o")
                v2[p] = (v_ps, as_ps)
            v_ps, as_ps = v2[p]
            if b == B_PER_CORE - 1:
                for i in range(8):
                    nc.tensor.matmul(
                        as_ps[K * h : K * (h + 1), :],
                        a_sb[:, i, :],
                        invz_bf[:, i : i + 1],
                        start=(i == 0),
                        stop=(i == 7),
                        skip_group_check=True,
                    )
            for i in range(8):
                nc.tensor.matmul(
                    v_ps[K * h : K * (h + 1), :],
                    a_sb[:, i, :],
                    xsg[i // 4][:, i % 4, :],
                    start=(i == 0),
                    stop=(i == 7),
                    skip_group_check=True,
                )
            if b != B_PER_CORE - 1:
                for i in range(8):
                    nc.tensor.matmul(
                        as_ps[K * h : K * (h + 1), :],
                        a_sb[:, i, :],
                        invz_bf[:, i : i + 1],
                        start=(i == 0),
                        stop=(i == 7),
                        skip_group_check=True,
                    )

            # -- pair complete: v = vT + asum*C^T; S_k = sum_d v^2 --
            if h == 1:
                asum = nrm.tile([128, 1], F32, tag="asum")
                nc.vector.tensor_copy(asum[:], as_ps[:])
                vc = nrm.tile([128, D], F32, tag="vc")
                nc.vector.tensor_scalar_mul(vc[:], ct2[:], asum[:])
                vv = nrm.tile([128, D], F32, tag=f"vv{p}")
                nc.vector.tensor_add(vv[:], vc[:], v_ps[:])
                v2[p] = vv
                sq = nrm.tile([128, D], F32, tag="sq")
                nc.vector.tensor_mul(sq[:], vv[:], vv[:])
                nc.vector.reduce_sum(S_all[:, p : p + 1], sq[:], axis=AX.X)

        # ---- norm tail: sc = 1/(8*sqrt(S+eps)) (global norm folded; gss==64) ----
        q8 = nrm.tile([128, 2], F32, tag="q8")
        nc.scalar.activation(q8[:], S_all[:], ACTF.Sqrt, bias=eps64_sb[:], scale=64.0)
        sc2 = nrm.tile([128, 2], F32, tag="sc2")
        nc.vector.reciprocal(sc2[:], q8[:])
        for p in range(2):
            vf = nrm.tile([128, D], F32, tag="vf")
            nc.vector.tensor_scalar_mul(vf[:], v2[p][:], sc2[:, p : p + 1])
            for hh in range(2):
                bb_i = 2 * p + hh
                o_ps = ps_o.tile([128, 4, K], F32, tag="o")
                for j in range(4):
                    nc.tensor.transpose(
                        o_ps[:, j, :],
                        vf[K * hh : K * (hh + 1), j * 128 : (j + 1) * 128],
                        id_f32[K * hh : K * (hh + 1), K * hh : K * (hh + 1)],
                    )
                o_sb = nrm.tile([128, 4, K], F32, tag="osb")
                nc.scalar.copy(o_sb[:], o_ps[:])
                nc.sync.dma_start(
                    out=out[bb_i].rearrange("(j p k) -> p j k", j=4, p=128, k=K),
                    in_=o_sb[:],
                )

    nc.compile()
    return nc


_CACHED_NC = None


def _get_nc():
    global _CACHED_NC
    if _CACHED_NC is None:
        _CACHED_NC = build_kernel()
    return _CACHED_NC


def build_in_maps(x, Wk, b, C):
    import ml_dtypes

    B = x.shape[0]
    x2 = np.ascontiguousarray(x, dtype=np.float32).reshape(B, N, D)
    bpc = B // N_CORES
    eye = np.eye(128)
    Wkf = np.asarray(Wk, dtype=np.float32)
    Cf = np.asarray(C, dtype=np.float32)
    bf = np.asarray(b, dtype=np.float32).reshape(K)
    consts = {
        "idbf": eye.astype(ml_dtypes.bfloat16),
        "idf": eye.astype(np.float32),
        "wkb": np.ascontiguousarray(
            Wkf.reshape(4, 128, K).transpose(1, 0, 2)
        ).astype(ml_dtypes.bfloat16),
        "ct2": np.ascontiguousarray(np.concatenate([Cf.T, Cf.T], axis=0)),
        "b2": np.concatenate([bf, bf]).reshape(128, 1),
        "iddbl": np.ascontiguousarray(
            np.concatenate([np.eye(K), np.eye(K)], axis=0).astype(np.float32)
        ),
    }
    in_maps = []
    for c in range(N_CORES):
        in_maps.append(
            {"x": x2[c * bpc : (c + 1) * bpc].reshape(bpc * N, D), **consts}
        )
    return in_maps


def kernel(x, Wk, b, C):
    """Full-input NetVLAD forward. x (32,32,32,512) f32 -> out (32, 32768) f32."""
    in_maps = build_in_maps(x, Wk, b, C)
    nc = _get_nc()
    res = run_bass_kernel_spmd(nc, in_maps, list(range(N_CORES)))
    return np.concatenate([res.results[c]["out"] for c in range(N_CORES)], axis=0)



# revision 4
# speedup vs baseline: 1.2382x; 1.2382x over previous
"""NetVLAD forward kernel for 8 TRN2 NeuronCores (Bass/Tile).

Reference (per batch b of 32):
  s = x @ Wk + b         (1024, 64) logits;  softmax over k -> a
  v[d,k] = sum_n a[n,k] x[n,d] + (sum_n a[n,k]) * C[d,k]
  v /= ||v||_2 over d (per k);  out = flatten(v) / ||flatten(v)||_2

Sharding: data-parallel over batch B=32 across 8 cores (4 batches/core).
Wk, b, C replicated; no collectives; host concatenates outputs.

Key layout/precision tricks vs the f32-input baseline:
  - x is uploaded in BOTH layouts (natural [n,d] for the aggregation
    matmul's moving operand, pre-transposed [d,n] for the logits matmul)
    as fp8 e3m4 -> 4MB HBM/core, and ZERO on-chip transposes of x
    (the baseline burned 128 PE matmuls/core transposing x).
  - Wk is host-prescaled by 64 (fp8 denormal avoidance); undone for free
    by the Exp activation's scale=1/64.
  - the e-transpose back to [n,k] is a regular matmul against an extended
    identity [I | g0col | g1col]; the two extra columns compute the
    softmax denominators Z (scaled by 1/64) in the same instruction.
  - softmax normalization is folded into a (8 small [128,64] DVE muls)
    instead of into x (8 big [128,512] muls); a stored as 64*a in fp8.
    All scale factors are powers of two and cancel in the L2 norms.
  - the final v^T transposes run as bf16 regular matmuls (64-col streams).
Engines: PE = matmuls only (~8.7K cols/batch), ACT = Exp + PSUM copies,
DVE = reciprocals/scaling/norm tail, sync+scalar issue HWDGE DMAs.
"""

import sys

sys.path.insert(0, "/opt/trn_rl_repo")

from contextlib import ExitStack

import numpy as np

import concourse.bacc as bacc
import concourse.tile as tile
from concourse import mybir
from concourse.bass_utils import run_bass_kernel_spmd

F32 = mybir.dt.float32
BF16 = mybir.dt.bfloat16
AX = mybir.AxisListType
OP = mybir.AluOpType
ACTF = mybir.ActivationFunctionType

B_PER_CORE = 4  # 32 batches / 8 cores
N = 1024  # H*W pixels per batch
D = 512
K = 64
EPS = 1e-12
N_CORES = 8

USE_FP8 = True
XDT = mybir.dt.float8e3 if USE_FP8 else BF16  # x / Wk / a storage dtype
WS = 64.0 if USE_FP8 else 1.0  # host pre-scale on Wk (denormal avoidance)
AS = 64.0 if USE_FP8 else 1.0  # on-chip scale on a (denormal avoidance)


def build_kernel():
    nc = bacc.Bacc()
    # [p, 4b+j, n]: xT[d=128j+p, n] per batch
    xt_d = nc.declare_dram_parameter("xt", [128, 4 * B_PER_CORE, N], XDT, isOutput=False)
    # [p, 8b+i, d]: x[n=128i+p, d] per batch
    xn_d = nc.declare_dram_parameter("xn", [128, 8 * B_PER_CORE, D], XDT, isOutput=False)
    out = nc.declare_dram_parameter("out", [B_PER_CORE, D * K], F32, isOutput=True)
    wkb_d = nc.declare_dram_parameter("wkb", [128, 4, K], XDT, isOutput=False)  # WS*Wk [p,j,k]
    idext_d = nc.declare_dram_parameter("idext", [128, 130], BF16, isOutput=False)
    id64_d = nc.declare_dram_parameter("id64", [128, K], BF16, isOutput=False)
    ct2_d = nc.declare_dram_parameter("ct2", [128, D], BF16, isOutput=False)  # [C^T; C^T]
    b2_d = nc.declare_dram_parameter("b2", [128, 1], F32, isOutput=False)  # [b; b]

    with tile.TileContext(nc) as tc, ExitStack() as ctx:
        const = ctx.enter_context(tc.tile_pool(name="const", bufs=1))
        xtp = ctx.enter_context(tc.tile_pool(name="xtp", bufs=4))
        xnp = ctx.enter_context(tc.tile_pool(name="xnp", bufs=4))
        sbm = ctx.enter_context(tc.tile_pool(name="sbm", bufs=2))
        nrm = ctx.enter_context(tc.tile_pool(name="nrm", bufs=2))
        # PSUM: s2 + a2 + v2 + as1 = 7 banks; o reuses the s pool
        ps_s = ctx.enter_context(tc.tile_pool(name="ps_s", bufs=2, space="PSUM"))
        ps_a = ctx.enter_context(tc.tile_pool(name="ps_a", bufs=2, space="PSUM"))
        ps_v = ctx.enter_context(tc.tile_pool(name="ps_v", bufs=2, space="PSUM"))
        ps_o = ctx.enter_context(tc.tile_pool(name="ps_o", bufs=1, space="PSUM"))

        # ---- constants (scalar/ACT engine HWDGE, parallel with x on sync) ----
        wkb = const.tile([128, 4, K], XDT)
        nc.scalar.dma_start(out=wkb[:], in_=wkb_d[:])
        idext = const.tile([128, 130], BF16)
        nc.scalar.dma_start(out=idext[:], in_=idext_d[:])
        b2_sb = const.tile([128, 1], F32)
        nc.scalar.dma_start(out=b2_sb[:], in_=b2_d[:])
        id64 = const.tile([128, K], BF16)
        nc.scalar.dma_start(out=id64[:], in_=id64_d[:])
        ct2 = const.tile([128, D], BF16)
        nc.scalar.dma_start(out=ct2[:], in_=ct2_d[:])
        ones_col = const.tile([128, 1], XDT)
        nc.vector.memset(ones_col[:], 1.0)
        eps64_sb = const.tile([128, 1], F32)
        nc.vector.memset(eps64_sb[:], float(64 * EPS))

        # ---- x loads: batch-interleaved so batch 0 lands first ----
        xts, xns = [], []
        for b in range(B_PER_CORE):
            xt_t = xtp.tile([128, 4, N], XDT, tag=f"xt{b}")
            xn_t = xnp.tile([128, 8, D], XDT, tag=f"xn{b}")
            # split xT by n-group so mm1(g=0) can start after 256KB
            nc.sync.dma_start(out=xt_t[:, :, 0:512], in_=xt_d[:, 4 * b : 4 * b + 4, 0:512])
            nc.sync.dma_start(
                out=xt_t[:, :, 512:1024], in_=xt_d[:, 4 * b : 4 * b + 4, 512:1024]
            )
            nc.sync.dma_start(out=xn_t[:], in_=xn_d[:, 8 * b : 8 * b + 8, :])
            xts.append(xt_t)
            xns.append(xn_t)

        as2 = ps_o.tile([128, 2], F32, tag="as2")
        S_all = nrm.tile([128, 2], F32, tag="sall")
        v2 = {}
        vv2 = {}
        for b in range(B_PER_CORE):
            p, h = b // 2, b % 2

            # -- mm1: s64^T[2g*64k, n] = sum_d (WS*Wk)[d,k] x^T[d,n] --
            s_ps = ps_s.tile([128, 512], F32, tag="s")
            for g in range(2):
                for j in range(4):
                    nc.tensor.matmul(
                        s_ps[K * g : K * (g + 1), :],
                        wkb[:, j, :],
                        xts[b][:, j, 512 * g : 512 * (g + 1)],
                        start=(j == 0),
                        stop=(j == 3),
                        skip_group_check=True,
                    )

            # -- exp(s + b) for both groups; WS undone via scale --
            eT = sbm.tile([128, 512], BF16, tag="eT")
            nc.scalar.activation(eT[:], s_ps[:], ACTF.Exp, bias=b2_sb[:], scale=1.0 / WS)

            # -- transpose e to [n,k] + Z/AS via extended identity; fold 1/Z --
            a_sb = sbm.tile([128, 4, 128], XDT, tag="a")
            for c in range(4):
                a_ps = ps_a.tile([128, 130], F32, tag="aps")
                nc.tensor.matmul(
                    a_ps[:],
                    eT[:, 128 * c : 128 * (c + 1)],
                    idext[:],
                    start=True,
                    stop=True,
                    skip_group_check=True,
                )
                iv = sbm.tile([128, 2], F32, tag="iv")
                nc.vector.reciprocal(iv[:], a_ps[:, 128:130])
                for g in range(2):
                    nc.vector.tensor_scalar_mul(
                        a_sb[:, c, K * g : K * (g + 1)],
                        a_ps[:, K * g : K * (g + 1)],
                        iv[:, g : g + 1],
                    )

            # -- mm2 + asum for the batch pair into [128, *] PSUM --
            if h == 0:
                v_ps = ps_v.tile([128, 512], F32, tag="v")
                v2[p] = v_ps
            v_ps = v2[p]
            for i in range(8):
                g, c = i // 4, i % 4
                nc.tensor.matmul(
                    v_ps[K * h : K * (h + 1), :],
                    a_sb[:, c, K * g : K * (g + 1)],
                    xns[b][:, i, :],
                    start=(i == 0),
                    stop=(i == 7),
                    skip_group_check=True,
                )
            for i in range(8):
                g, c = i // 4, i % 4
                nc.tensor.matmul(
                    as2[K * h : K * (h + 1), p : p + 1],
                    a_sb[:, c, K * g : K * (g + 1)],
                    ones_col[:],
                    start=(i == 0),
                    stop=(i == 7),
                    skip_group_check=True,
                )

            # -- pair complete: v = vT + asum*C^T; S_k = sum_d v^2 --
            if h == 1:
                asum = nrm.tile([128, 1], F32, tag="asum")
                nc.vector.tensor_copy(asum[:], as2[:, p : p + 1])
                vc = nrm.tile([128, D], F32, tag="vc")
                nc.vector.tensor_scalar_mul(vc[:], ct2[:], asum[:])
                vv = nrm.tile([128, D], F32, tag=f"vv{p}")
                nc.vector.tensor_add(vv[:], vc[:], v_ps[:])
                vv2[p] = vv
                sq = nrm.tile([128, D], F32, tag="sq")
                nc.vector.tensor_mul(sq[:], vv[:], vv[:])
                nc.vector.reduce_sum(S_all[:, p : p + 1], sq[:], axis=AX.X)

        # ---- norm tail: sc = 1/(8*sqrt(S+eps)) (global norm folded; gss==64) ----
        q8 = nrm.tile([128, 2], F32, tag="q8")
        nc.scalar.activation(q8[:], S_all[:], ACTF.Sqrt, bias=eps64_sb[:], scale=64.0)
        sc2 = nrm.tile([128, 2], F32, tag="sc2")
        nc.vector.reciprocal(sc2[:], q8[:])
        for p in range(2):
            vf = nrm.tile([128, D], BF16, tag="vf")
            nc.vector.tensor_scalar_mul(vf[:], vv2[p][:], sc2[:, p : p + 1])
            for hh in range(2):
                bb_i = 2 * p + hh
                o_ps = ps_s.tile([128, 4, K], F32, tag="s")
                for j in range(4):
                    nc.tensor.matmul(
                        o_ps[:, j, :],
                        vf[K * hh : K * (hh + 1), j * 128 : (j + 1) * 128],
                        id64[K * hh : K * (hh + 1), :],
                        start=True,
                        stop=True,
                        skip_group_check=True,
                    )
                o_sb = nrm.tile([128, 4, K], F32, tag="osb")
                nc.scalar.copy(o_sb[:], o_ps[:])
                nc.sync.dma_start(
                    out=out[bb_i].rearrange("(j p k) -> p j k", j=4, p=128, k=K),
                    in_=o_sb[:],
                )

    nc.compile()
    return nc


_CACHED_NC = None


def _get_nc():
    global _CACHED_NC
    if _CACHED_NC is None:
        _CACHED_NC = build_kernel()
    return _CACHED_NC


def build_in_maps(x, Wk, b, C):
    import ml_dtypes

    XNP = ml_dtypes.float8_e3m4 if USE_FP8 else ml_dtypes.bfloat16
    B = x.shape[0]
    x2 = np.ascontiguousarray(x, dtype=np.float32).reshape(B, N, D)
    bpc = B // N_CORES
    Wkf = np.asarray(Wk, dtype=np.float32)
    Cf = np.asarray(C, dtype=np.float32)
    bf = np.asarray(b, dtype=np.float32).reshape(K)
    idext = np.zeros((128, 130), dtype=np.float32)
    idext[:, :128] = np.eye(128)
    idext[0:64, 128] = 1.0 / AS
    idext[64:128, 129] = 1.0 / AS
    consts = {
        "wkb": np.ascontiguousarray(
            (Wkf * WS).reshape(4, 128, K).transpose(1, 0, 2)
        ).astype(XNP),
        "idext": idext.astype(ml_dtypes.bfloat16),
        "id64": np.concatenate([np.eye(K), np.eye(K)], axis=0).astype(
            ml_dtypes.bfloat16
        ),
        "ct2": np.concatenate([Cf.T, Cf.T], axis=0).astype(ml_dtypes.bfloat16),
        "b2": np.concatenate([bf, bf]).reshape(128, 1),
    }
    in_maps = []
    for c in range(N_CORES):
        xc = x2[c * bpc : (c + 1) * bpc]  # [4, 1024, 512]
        xn = np.ascontiguousarray(
            xc.reshape(bpc, 8, 128, D).transpose(2, 0, 1, 3).reshape(128, 8 * bpc, D)
        ).astype(XNP)
        xt = np.ascontiguousarray(
            xc.transpose(2, 0, 1)
            .reshape(4, 128, bpc, N)
            .transpose(1, 2, 0, 3)
            .reshape(128, 4 * bpc, N)
        ).astype(XNP)
        in_maps.append({"xt": xt, "xn": xn, **consts})
    return in_maps


def kernel(x, Wk, b, C):
    """Full-input NetVLAD forward. x (32,32,32,512) f32 -> out (32, 32768) f32."""
    in_maps = build_in_maps(x, Wk, b, C)
    nc = _get_nc()
    res = run_bass_kernel_spmd(nc, in_maps, list(range(N_CORES)))
    return np.concatenate([res.results[c]["out"] for c in range(N_CORES)], axis=0)
